# revision 1
# baseline (speedup 1.0000x reference)
"""AttentionSeq2Seq Trainium kernel: DP-8, fully unrolled raw bass.

Per core (batch slice of 8):
  - encoder LSTM TENC steps, decoder LSTM TDEC steps, all local.
  - gates psum[128, 32 m-chunks x 8] accumulated from: 8 h-chunk MMs (whhT
    bf16 stationary) + 1 vocab one-hot MM (gxv table = embed@W_ih^T + bias).
  - gate rows host-permuted to [i, f, o, g] so one sigmoid covers i,f,o.
  - context = sum_t enc_h accumulated on DVE; decoder adds
    ctx_term = W_ihH @ ctx to the pre-activations each step (DVE TT add).
  - out[t] = h @ out_W^T as 8 small MMs per decoder step into psum[8,15],
    copied to a 64-step sbuf ring by ACT, ring flushed to DRAM by gpsimd.
  - whhT sbuf buffer reloaded with dec_W_hh between phases.
"""
import sys
sys.path.insert(0, '/opt/trn_rl_repo')
import numpy as np
import concourse.bass as bass
import concourse.mybir as mybir

F32 = mybir.dt.float32
BF16 = mybir.dt.float16  # NB: 16-bit compute dtype (fp16: 3 more mantissa bits than bf16, same FWL speed)
AF = mybir.ActivationFunctionType
AL = mybir.AluOpType
NPBF16 = mybir.dt.np(BF16)

H = 1024
E = 512
O = 15
SRC_V = 64
TGT_V = 16
KCH = 8           # h contraction chunks (1024/128)
MCH = 32          # gate-row chunks (4096/128)
B = 8             # batch per core


def build_nc(TENC=512, TDEC=512, CH=128, OBLK=64):
    OBLK = min(OBLK, TDEC)
    assert TENC % CH == 0 and TDEC % CH == 0 and TDEC % OBLK == 0
    nc = bass.Bass(target_bir_lowering=False, debug=False)

    whhT_d = nc.declare_dram_parameter("whhT", [128, KCH * 4096], BF16, isOutput=False)
    dwhhT_d = nc.declare_dram_parameter("dwhhT", [128, KCH * 4096], BF16, isOutput=False)
    dihHT_d = nc.declare_dram_parameter("dihHT", [128, KCH * 4096], BF16, isOutput=False)
    ihETe_d = nc.declare_dram_parameter("ihETe", [128, 4 * 4096], BF16, isOutput=False)
    ihETd_d = nc.declare_dram_parameter("ihETd", [128, 4 * 4096], BF16, isOutput=False)
    embTe_d = nc.declare_dram_parameter("embTe", [128, 4 * SRC_V], BF16, isOutput=False)
    embTd_d = nc.declare_dram_parameter("embTd", [128, 4 * TGT_V], BF16, isOutput=False)
    bvece_d = nc.declare_dram_parameter("bvece", [1, 4096], BF16, isOutput=False)
    bvecd_d = nc.declare_dram_parameter("bvecd", [1, 4096], BF16, isOutput=False)
    onesv_d = nc.declare_dram_parameter("onesv", [1, SRC_V], BF16, isOutput=False)
    outWT_d = nc.declare_dram_parameter("outWT", [128, KCH * O], BF16, isOutput=False)
    ohe_d = nc.declare_dram_parameter("ohe", [SRC_V, TENC * B], BF16, isOutput=False)
    ohd_d = nc.declare_dram_parameter("ohd", [TGT_V, TDEC * B], BF16, isOutput=False)
    out_d = nc.declare_dram_parameter("out", [B, TDEC * O], F32, isOutput=True)

    NECH = TENC // CH
    NDCH = TDEC // CH
    NOB = TDEC // OBLK
    Q = OBLK * O      # out ring block width (f32 elems per row)
    GW = MCH * B      # 256: gates free width
    BK = 512          # psum bank stride (f32 elems)

    # DVE op-count formulas (s_dv): 16 setup copies; enc step ops
    # [tmp1,tmp2,cst,hf,ctx,hT]; ctxb; dec step ops [S2,tmp1,tmp2,cst,hf,hT]
    dv_e = lambda t: 16 + 5 * t
    dv_ctxb = 16 + 5 * TENC + 1
    dv_d = lambda t: dv_ctxb + 5 * t
    # ACT op-count formulas (s_ac): enc [sig,tg,tc]; ctxterm; dec [sig,tg,tc]
    ac_e = lambda t: 3 * t
    ac_ctxt = 3 * TENC + 1
    ac_d = lambda t: ac_ctxt + 3 * t

    from contextlib import ExitStack
    with ExitStack() as _es:
        ec = _es.enter_context
        block = ec(nc.Block())
        s_d1 = ec(nc.semaphore("s_d1"))
        s_d2 = ec(nc.semaphore("s_d2"))
        s_d3 = ec(nc.semaphore("s_d3"))
        s_dwh = ec(nc.semaphore("s_dwh"))
        s_dwh2 = ec(nc.semaphore("s_dwh2"))
        s_dih = ec(nc.semaphore("s_dih"))
        s_dow = ec(nc.semaphore("s_dow"))
        s_de = [ec(nc.semaphore("s_de0")), ec(nc.semaphore("s_de1"))]
        s_dd = [ec(nc.semaphore("s_dd0")), ec(nc.semaphore("s_dd1"))]
        s_od = [ec(nc.semaphore("s_od0")), ec(nc.semaphore("s_od1"))]
        s_init = ec(nc.semaphore("s_init"))
        s_gx = ec(nc.semaphore("s_gx"))
        s_gv = ec(nc.semaphore("s_gv"))
        s_pe = ec(nc.semaphore("s_pe"))
        s_peo = ec(nc.semaphore("s_peo"))
        s_oc = ec(nc.semaphore("s_oc"))
        s_ctm = ec(nc.semaphore("s_ctm"))
        s_dv = ec(nc.semaphore("s_dv"))
        s_ac = ec(nc.semaphore("s_ac"))
        whhT = ec(nc.sbuf_tensor("whhT_s", [128, KCH * 4096], BF16))
        dihHT = ec(nc.sbuf_tensor("dihHT_s", [128, KCH * 4096], BF16))
        scratch = ec(nc.sbuf_tensor("scratch_s", [128, 4 * 4096], BF16))
        outsb = scratch[0:B, 0:4 * Q].bitcast(F32)  # out ring reuses dead setup scratch
        gxve = ec(nc.sbuf_tensor("gxve_s", [SRC_V, 4096], BF16))
        gxvd = ec(nc.sbuf_tensor("gxvd_s", [TGT_V, 4096], BF16))
        embTe = ec(nc.sbuf_tensor("embTe_s", [128, 4 * SRC_V], BF16))
        embTd = ec(nc.sbuf_tensor("embTd_s", [128, 4 * TGT_V], BF16))
        bvece = ec(nc.sbuf_tensor("bvece_s", [1, 4096], BF16))
        bvecd = ec(nc.sbuf_tensor("bvecd_s", [1, 4096], BF16))
        onesv = ec(nc.sbuf_tensor("onesv_s", [1, SRC_V], BF16))
        outWT = ec(nc.sbuf_tensor("outWT_s", [128, KCH * O], BF16))
        ohe = ec(nc.sbuf_tensor("ohe_s", [SRC_V, 2 * CH * B], BF16))
        ohd = ec(nc.sbuf_tensor("ohd_s", [TGT_V, 2 * CH * B], BF16))
        hT = ec(nc.sbuf_tensor("hT_s", [128, KCH * B], BF16))
        cst = ec(nc.sbuf_tensor("cst_s", [128, KCH * B], F32))
        S = ec(nc.sbuf_tensor("S_s", [128, GW], F32))
        S2 = ec(nc.sbuf_tensor("S2_s", [128, GW], F32))
        tmp1 = ec(nc.sbuf_tensor("tmp1_s", [128, KCH * B], F32))
        tmp2 = ec(nc.sbuf_tensor("tmp2_s", [128, KCH * B], F32))
        tcs = ec(nc.sbuf_tensor("tcs_s", [128, KCH * B], F32))
        hf = ec(nc.sbuf_tensor("hf_s", [128, KCH * B], F32))
        ctx = ec(nc.sbuf_tensor("ctx_s", [128, KCH * B], F32))
        ctxb = ec(nc.sbuf_tensor("ctxb_s", [128, KCH * B], BF16))
        ctxterm = ec(nc.sbuf_tensor("ctxterm_s", [128, GW], F32))
        ps_g = ec(nc.psum_tensor("ps_g_s", [128, 2 * BK], F32))
        ps_o = ec(nc.psum_tensor("ps_o_s", [B, 2 * BK], F32))
        ps_x = ec(nc.psum_tensor("ps_x_s", [SRC_V, BK], F32))
        ps_ct = ec(nc.psum_tensor("ps_ct_s", [128, GW], F32))
        # ============ GPSIMD: init memsets + out ring flushes ============
        @block.gpsimd
        def _(gp):
            gp.memset(hT[:, :], 0.0).then_inc(s_init, 1)
            gp.memset(cst[:, :], 0.0).then_inc(s_init, 1)
            gp.memset(ctx[:, :], 0.0).then_inc(s_init, 1)
            for b in range(NOB):
                gp.wait_ge(s_oc, (b + 1) * OBLK)
                gp.dma_start(out=out_d[:, b * Q:(b + 1) * Q],
                             in_=outsb[:, (b % 2) * Q:(b % 2) * Q + Q]
                             ).then_inc(s_od[b % 2], 16)
            gp.wait_ge(s_od[0], 16 * ((NOB + 1) // 2))
            if NOB > 1:
                gp.wait_ge(s_od[1], 16 * (NOB // 2))

        # ============ SYNC: input DMAs ============
        @block.sync
        def _(sy):
            def dma(dst, src, sem):
                sy.dma_start(out=dst, in_=src).then_inc(sem, 16)

            dma(embTe[:, :], embTe_d[:, :], s_d1)
            dma(scratch[:, :], ihETe_d[:, :], s_d1)
            dma(bvece[:, :], bvece_d[:, :], s_d1)
            dma(onesv[:, :], onesv_d[:, :], s_d1)
            dma(embTd[:, :], embTd_d[:, :], s_d2)
            dma(bvecd[:, :], bvecd_d[:, :], s_d2)
            dma(ohe[:, 0:CH * B], ohe_d[:, 0:CH * B], s_de[0])
            dma(whhT[:, :], whhT_d[:, :], s_dwh)
            dma(outWT[:, :], outWT_d[:, :], s_dow)
            dma(ohd[:, 0:CH * B], ohd_d[:, 0:CH * B], s_dd[0])
            dma(dihHT[:, :], dihHT_d[:, :], s_dih)
            sy.wait_ge(s_gx, 8)  # scratch free after enc gxv MMs
            dma(scratch[:, :], ihETd_d[:, :], s_d3)
            for c in range(1, NECH):
                if c >= 2:
                    sy.wait_ge(s_pe, (c - 1) * CH)
                dma(ohe[:, (c % 2) * CH * B:(c % 2 + 1) * CH * B],
                    ohe_d[:, c * CH * B:(c + 1) * CH * B], s_de[c % 2])
            # decoder recurrence weights replace encoder's
            sy.wait_ge(s_pe, TENC)
            dma(whhT[:, :], dwhhT_d[:, :], s_dwh2)
            for c in range(1, NDCH):
                if c >= 2:
                    sy.wait_ge(s_pe, TENC + (c - 1) * CH)
                dma(ohd[:, (c % 2) * CH * B:(c % 2 + 1) * CH * B],
                    ohd_d[:, c * CH * B:(c + 1) * CH * B], s_dd[c % 2])

        # ============ TENSOR ============
        @block.tensor
        def _(te):
            te.wait_ge(s_d1, 64)
            for n in range(8):
                if n > 0:
                    te.wait_ge(s_dv, n)
                for k in range(4):
                    te.matmul(ps_x[:SRC_V, :],
                              embTe[:, k * SRC_V:(k + 1) * SRC_V],
                              scratch[:, k * 4096 + n * 512:k * 4096 + (n + 1) * 512],
                              start=(k == 0), stop=False)
                te.matmul(ps_x[:SRC_V, :], onesv[0:1, :],
                          bvece[0:1, n * 512:(n + 1) * 512],
                          start=False, stop=True).then_inc(s_gx, 1)
            te.wait_ge(s_d2, 32)
            te.wait_ge(s_d3, 16)
            for n in range(8):
                te.wait_ge(s_dv, 8 + n)
                for k in range(4):
                    te.matmul(ps_x[:TGT_V, :],
                              embTd[:, k * TGT_V:(k + 1) * TGT_V],
                              scratch[:, k * 4096 + n * 512:k * 4096 + (n + 1) * 512],
                              start=(k == 0), stop=False)
                te.matmul(ps_x[:TGT_V, :], onesv[0:1, 0:TGT_V],
                          bvecd[0:1, n * 512:(n + 1) * 512],
                          start=False, stop=True).then_inc(s_gx, 1)

            # ---- encoder ----
            te.wait_ge(s_dwh, 16)
            te.wait_ge(s_init, 2)
            te.wait_ge(s_dv, 16)
            for t in range(TENC):
                c = t // CH
                if t % CH == 0:
                    te.wait_ge(s_de[c % 2], 16 * (c // 2 + 1))
                if t > 0:
                    te.wait_ge(s_dv, dv_e(t - 1) + 4)
                if t >= 2:
                    te.wait_ge(s_ac, ac_e(t - 2) + 2)
                pb = ps_g[:, (t % 2) * BK:(t % 2) * BK + GW]
                ohs = ohe[:, ((c % 2) * CH + (t % CH)) * B:((c % 2) * CH + (t % CH)) * B + B]
                for m in range(MCH):
                    o = pb[:, m * B:(m + 1) * B]
                    for k in range(KCH):
                        te.matmul(o, whhT[:, k * 4096 + m * 128:k * 4096 + (m + 1) * 128],
                                  hT[:, k * B:(k + 1) * B], start=(k == 0), stop=False)
                    mm = te.matmul(o, gxve[:, m * 128:(m + 1) * 128], ohs,
                                   start=False, stop=True)
                mm.then_inc(s_pe, 1)

            # ---- ctx_term ----
            te.wait_ge(s_dih, 16)
            te.wait_ge(s_dv, dv_ctxb)
            for m in range(MCH):
                for k in range(KCH):
                    te.matmul(ps_ct[:, m * B:(m + 1) * B],
                              dihHT[:, k * 4096 + m * 128:k * 4096 + (m + 1) * 128],
                              ctxb[:, k * B:(k + 1) * B],
                              start=(k == 0), stop=(k == KCH - 1),
                              ).then_maybe_inc((s_ctm, 1) if k == KCH - 1 else None)

            # ---- decoder ----
            te.wait_ge(s_dow, 16)
            te.wait_ge(s_dwh2, 16)
            for t in range(TDEC):
                u = TENC + t
                c = t // CH
                if t % CH == 0:
                    te.wait_ge(s_dd[c % 2], 16 * (c // 2 + 1))
                if t == 0:
                    te.wait_ge(s_dv, dv_ctxb)
                else:
                    te.wait_ge(s_dv, dv_d(t - 1) + 5)
                if t > 0:
                    if t >= 3:
                        te.wait_ge(s_oc, t - 2)
                    for k in range(KCH):
                        te.matmul(ps_o[:, ((t - 1) % 2) * BK:((t - 1) % 2) * BK + O],
                                  hT[:, k * B:(k + 1) * B],
                                  outWT[:, k * O:(k + 1) * O],
                                  start=(k == 0), stop=(k == KCH - 1),
                                  ).then_maybe_inc((s_peo, 1) if k == KCH - 1 else None)
                if t < 2:
                    te.wait_ge(s_ac, ac_e(TENC - 2 + t) + 2)
                else:
                    te.wait_ge(s_dv, dv_d(t - 2) + 1)
                pb = ps_g[:, (u % 2) * BK:(u % 2) * BK + GW]
                ohs = ohd[:, ((c % 2) * CH + (t % CH)) * B:((c % 2) * CH + (t % CH)) * B + B]
                for m in range(MCH):
                    o = pb[:, m * B:(m + 1) * B]
                    for k in range(KCH):
                        te.matmul(o, whhT[:, k * 4096 + m * 128:k * 4096 + (m + 1) * 128],
                                  hT[:, k * B:(k + 1) * B], start=(k == 0), stop=False)
                    mm = te.matmul(o, gxvd[:, m * 128:(m + 1) * 128], ohs,
                                   start=False, stop=True)
                mm.then_inc(s_pe, 1)
            # tail out-MM
            te.wait_ge(s_dv, dv_d(TDEC - 1) + 5)
            te.wait_ge(s_oc, TDEC - 2)
            for k in range(KCH):
                te.matmul(ps_o[:, ((TDEC - 1) % 2) * BK:((TDEC - 1) % 2) * BK + O],
                          hT[:, k * B:(k + 1) * B], outWT[:, k * O:(k + 1) * O],
                          start=(k == 0), stop=(k == KCH - 1),
                          ).then_maybe_inc((s_peo, 1) if k == KCH - 1 else None)

        # ============ SCALAR (ACT) ============
        @block.scalar
        def _(ac):
            for t in range(TENC):
                pb = ps_g[:, (t % 2) * BK:(t % 2) * BK + GW]
                ac.wait_ge(s_pe, t + 1)
                ac.activation(S[:, 0:24 * B], pb[:, 0:24 * B], AF.Sigmoid).then_inc(s_ac, 1)
                ac.activation(S[:, 24 * B:32 * B], pb[:, 24 * B:32 * B], AF.Tanh
                              ).then_inc(s_ac, 1)
                ac.wait_ge(s_dv, dv_e(t) + 3)
                ac.activation(tcs[:, :], cst[:, :], AF.Tanh).then_inc(s_ac, 1)
            ac.wait_ge(s_ctm, MCH)
            ac.activation(ctxterm[:, :], ps_ct[:, :], AF.Copy).then_inc(s_ac, 1)
            for t in range(TDEC):
                ac.wait_ge(s_dv, dv_d(t) + 1)
                ac.activation(S[:, 0:24 * B], S2[:, 0:24 * B], AF.Sigmoid).then_inc(s_ac, 1)
                ac.activation(S[:, 24 * B:32 * B], S2[:, 24 * B:32 * B], AF.Tanh
                              ).then_inc(s_ac, 1)
                ac.wait_ge(s_dv, dv_d(t) + 4)
                ac.activation(tcs[:, :], cst[:, :], AF.Tanh).then_inc(s_ac, 1)
                if t > 0:
                    tb = t - 1
                    if tb >= 2 * OBLK:
                        bb = tb // OBLK
                        ac.wait_ge(s_od[bb % 2], 16 * ((bb - 2) // 2 + 1))
                    ac.wait_ge(s_peo, t)
                    ac.activation(outsb[:, ((tb // OBLK) % 2) * Q + (tb % OBLK) * O:
                                  ((tb // OBLK) % 2) * Q + (tb % OBLK) * O + O],
                                  ps_o[:, (tb % 2) * BK:(tb % 2) * BK + O],
                                  AF.Copy).then_inc(s_oc, 1)
            tb = TDEC - 1
            ac.wait_ge(s_peo, TDEC)
            ac.activation(outsb[:, ((tb // OBLK) % 2) * Q + (tb % OBLK) * O:
                          ((tb // OBLK) % 2) * Q + (tb % OBLK) * O + O],
                          ps_o[:, (tb % 2) * BK:(tb % 2) * BK + O],
                          AF.Copy).then_inc(s_oc, 1)

        # ============ VECTOR (DVE) ============
        @block.vector
        def _(v):
            for n in range(8):
                v.wait_ge(s_gx, n + 1)
                v.tensor_copy(gxve[:, n * 512:(n + 1) * 512], ps_x[:SRC_V, :]
                              ).then_inc(s_dv, 1)
            for n in range(8):
                v.wait_ge(s_gx, 8 + n + 1)
                v.tensor_copy(gxvd[:, n * 512:(n + 1) * 512], ps_x[:TGT_V, :]
                              ).then_inc(s_dv, 1)
            v.wait_ge(s_init, 3)
            for t in range(TENC):
                v.wait_ge(s_ac, ac_e(t) + 2)
                v.tensor_tensor(tmp1[:, :], S[:, 8 * B:16 * B], cst[:, :], AL.mult
                                ).then_inc(s_dv, 1)
                v.tensor_tensor(tmp2[:, :], S[:, 0:8 * B], S[:, 24 * B:32 * B], AL.mult
                                ).then_inc(s_dv, 1)
                v.wait_ge(s_dv, dv_e(t) + 2)
                v.tensor_tensor(cst[:, :], tmp1[:, :], tmp2[:, :], AL.add
                                ).then_inc(s_dv, 1)
                v.wait_ge(s_ac, ac_e(t) + 3)
                v.tensor_tensor(hT[:, :], S[:, 16 * B:24 * B], tcs[:, :], AL.mult
                                ).then_inc(s_dv, 1)
                v.wait_ge(s_dv, dv_e(t) + 4)
                v.tensor_tensor(ctx[:, :], ctx[:, :], hT[:, :], AL.add).then_inc(s_dv, 1)
            v.wait_ge(s_dv, dv_e(TENC - 1) + 5)
            v.tensor_copy(ctxb[:, :], ctx[:, :]).then_inc(s_dv, 1)
            for t in range(TDEC):
                u = TENC + t
                pb = ps_g[:, (u % 2) * BK:(u % 2) * BK + GW]
                v.wait_ge(s_pe, u + 1)
                if t == 0:
                    v.wait_ge(s_ac, ac_ctxt)
                v.tensor_tensor(S2[:, :], pb[:, :], ctxterm[:, :], AL.add
                                ).then_inc(s_dv, 1)
                v.wait_ge(s_ac, ac_d(t) + 2)
                v.tensor_tensor(tmp1[:, :], S[:, 8 * B:16 * B], cst[:, :], AL.mult
                                ).then_inc(s_dv, 1)
                v.tensor_tensor(tmp2[:, :], S[:, 0:8 * B], S[:, 24 * B:32 * B], AL.mult
                                ).then_inc(s_dv, 1)
                v.wait_ge(s_dv, dv_d(t) + 3)
                v.tensor_tensor(cst[:, :], tmp1[:, :], tmp2[:, :], AL.add
                                ).then_inc(s_dv, 1)
                v.wait_ge(s_ac, ac_d(t) + 3)
                v.tensor_tensor(hT[:, :], S[:, 16 * B:24 * B], tcs[:, :], AL.mult
                                ).then_inc(s_dv, 1)

    return nc


def prep_inputs(inp, TENC=512, TDEC=512):
    perm = np.concatenate([np.arange(0, H), np.arange(H, 2 * H),
                           np.arange(3 * H, 4 * H), np.arange(2 * H, 3 * H)])
    f32 = lambda x: np.asarray(x, np.float32)

    W_hh = f32(inp["enc_W_hh"])[perm]
    dW_hh = f32(inp["dec_W_hh"])[perm]
    W_ihE = f32(inp["enc_W_ih"])[perm]
    dW_ih = f32(inp["dec_W_ih"])[perm]
    dW_ihE = dW_ih[:, :E]
    dW_ihH = dW_ih[:, E:]
    b_e = (f32(inp["enc_b_ih"]) + f32(inp["enc_b_hh"]))[perm]
    b_d = (f32(inp["dec_b_ih"]) + f32(inp["dec_b_hh"]))[perm]
    out_W = f32(inp["out_W"])
    out_b = f32(inp["out_b"])
    embE = f32(inp["enc_embed"])
    embD = f32(inp["dec_embed"])
    ei = np.asarray(inp["encoder_inputs"]).astype(np.int64)
    di = np.asarray(inp["decoder_inputs"]).astype(np.int64)

    def kmaj(Wm, K):
        R = Wm.shape[0]
        outp = np.empty((128, K * R), np.float32)
        for k in range(K):
            outp[:, k * R:(k + 1) * R] = Wm[:, k * 128:(k + 1) * 128].T
        return outp

    common = {
        "whhT": kmaj(W_hh, KCH).astype(NPBF16),
        "dwhhT": kmaj(dW_hh, KCH).astype(NPBF16),
        "dihHT": kmaj(dW_ihH, KCH).astype(NPBF16),
        "ihETe": kmaj(W_ihE, 4).astype(NPBF16),
        "ihETd": kmaj(dW_ihE, 4).astype(NPBF16),
        "embTe": kmaj(embE, 4).astype(NPBF16),
        "embTd": kmaj(embD, 4).astype(NPBF16),
        "bvece": b_e[None, :].astype(NPBF16),
        "bvecd": b_d[None, :].astype(NPBF16),
        "onesv": np.ones((1, SRC_V), NPBF16),
        "outWT": kmaj(out_W, KCH).astype(NPBF16),
    }
    in_maps = []
    for r in range(8):
        bs = slice(r * B, (r + 1) * B)
        eit = ei[bs, :TENC]
        dit = di[bs, :TDEC]
        ohe = (np.arange(SRC_V)[:, None, None] == eit.T[None, :, :]).astype(NPBF16)
        ohd = (np.arange(TGT_V)[:, None, None] == dit.T[None, :, :]).astype(NPBF16)
        in_maps.append(dict(common,
                            ohe=ohe.reshape(SRC_V, TENC * B),
                            ohd=ohd.reshape(TGT_V, TDEC * B)))
    return in_maps, out_b


def assemble(results, out_b, TDEC=512):
    outs = [np.asarray(r["out"], np.float32).reshape(B, TDEC, O) for r in results]
    return np.concatenate(outs, axis=0) + out_b[None, None, :]


# ======================== runner ========================
import time
import numpy as np
import jax
from jax.sharding import Mesh, PartitionSpec
from jax.experimental.shard_map import shard_map
import concourse.mybir as mybir
from concourse import bass2jax
from concourse.bass2jax import _bass_exec_p, install_neuronx_cc_hook, partition_id_tensor


class CompiledSpmd:
    def __init__(self, nc, n_cores=8):
        install_neuronx_cc_hook()
        self.nc = nc
        self.n_cores = n_cores
        partition_name = nc.partition_id_tensor.name if nc.partition_id_tensor else None
        in_names, out_names, out_avals = [], [], []
        for alloc in nc.m.functions[0].allocations:
            if not isinstance(alloc, mybir.MemoryLocationSet):
                continue
            name = alloc.memorylocations[0].name
            if alloc.kind == "ExternalInput":
                if name != partition_name:
                    in_names.append(name)
            elif alloc.kind == "ExternalOutput":
                shape = tuple(alloc.tensor_shape)
                dtype = mybir.dt.np(alloc.dtype)
                out_names.append(name)
                out_avals.append(jax.core.ShapedArray(shape, dtype))
        self.in_names = list(in_names)
        self.out_names = out_names
        self.out_avals = out_avals
        n_params = len(in_names)
        n_outs = len(out_avals)
        all_in_names = list(in_names) + list(out_names)
        if partition_name is not None:
            all_in_names.append(partition_name)
        self.partition_name = partition_name

        def _body(*args):
            operands = list(args)
            if partition_name is not None:
                operands.append(partition_id_tensor())
            outs = _bass_exec_p.bind(
                *operands,
                out_avals=tuple(out_avals),
                in_names=tuple(all_in_names),
                out_names=tuple(out_names),
                lowering_input_output_aliases=(),
                sim_require_finite=True,
                sim_require_nnan=True,
                nc=nc,
            )
            return tuple(outs)

        devices = jax.devices()[:n_cores]
        mesh = Mesh(np.asarray(devices), ("core",))
        self._mesh = mesh
        in_specs = (PartitionSpec("core"),) * (n_params + n_outs)
        out_specs = (PartitionSpec("core"),) * len(out_names)
        donate = tuple(range(n_params, n_params + n_outs))
        self._fn = jax.jit(
            shard_map(_body, mesh=mesh, in_specs=in_specs, out_specs=out_specs,
                      check_rep=False),
            donate_argnums=donate, keep_unused=True)
        self.n_params = n_params
        self.n_outs = n_outs

    def pack(self, in_maps):
        per_core = [[np.asarray(m[n]) for n in self.in_names] for m in in_maps]
        return [np.concatenate([per_core[c][i] for c in range(self.n_cores)], axis=0)
                for i in range(self.n_params)]

    def zeros(self):
        return [np.zeros((self.n_cores * a.shape[0], *a.shape[1:]), a.dtype)
                for a in self.out_avals]

    def run(self, concat_in):
        out = self._fn(*concat_in, *self.zeros())
        jax.block_until_ready(out)
        return out

    def results(self, out_arrs):
        return [
            {name: np.asarray(out_arrs[i]).reshape(self.n_cores, *self.out_avals[i].shape)[c]
             for i, name in enumerate(self.out_names)}
            for c in range(self.n_cores)
        ]

    def bench(self, in_maps, iters=6, warmup=2):
        ci = self.pack(in_maps)
        for _ in range(warmup):
            self.run(ci)
        ts = []
        for _ in range(iters):
            t0 = time.time()
            self.run(ci)
            ts.append(time.time() - t0)
        return min(ts), sorted(ts)[len(ts) // 2]

    def bench_pipelined(self, in_maps, n=20, warmup=2):
        """Queue n executions asynchronously, block once. Returns total/n."""
        import jax
        ci = self.pack(in_maps)
        for _ in range(warmup):
            self.run(ci)
        t0 = time.time()
        outs = []
        for _ in range(n):
            outs.append(self._fn(*ci, *self.zeros()))
        jax.block_until_ready(outs)
        return (time.time() - t0) / n

    def bench_resident(self, in_maps, n=10, warmup=2):
        """Device-resident inputs: isolates execution+dispatch from H2D."""
        import jax
        from jax.sharding import NamedSharding, PartitionSpec
        mesh = self._mesh
        sh = NamedSharding(mesh, PartitionSpec("core"))
        ci = [jax.device_put(x, sh) for x in self.pack(in_maps)]
        jax.block_until_ready(ci)
        for _ in range(warmup):
            jax.block_until_ready(self._fn(*ci, *self.zeros()))
        t0 = time.time()
        outs = []
        for _ in range(n):
            outs.append(self._fn(*ci, *self.zeros()))
        jax.block_until_ready(outs)
        return (time.time() - t0) / n


# ======================== public entry point ========================
_CACHE = {}


def kernel(**inputs):
    """Full-input, full-output AttentionSeq2Seq forward on 8 NeuronCores."""
    if "runner" not in _CACHE:
        nc = build_nc(TENC=512, TDEC=512, CH=128)
        _CACHE["runner"] = CompiledSpmd(nc, n_cores=8)
    r = _CACHE["runner"]
    in_maps, out_b = prep_inputs(inputs)
    outs = r.results(r.run(r.pack(in_maps)))
    return assemble(outs, out_b).astype(np.float32)

